# revision 4
# baseline (speedup 1.0000x reference)
import sys
sys.path.insert(0, '/opt/trn_rl_repo')
from contextlib import ExitStack

import numpy as np
import ml_dtypes

import concourse.hw_specs as _hw
import functools as _ft

_orig_get_tables = _hw.get_activation_tables


@_ft.cache
def _patched_tables(arch):
    # Keep dict order (act_func_set_id is positional); instead strip Exp
    # from tables lacking Ln so the load-insertion pass picks the table
    # that has both, avoiding per-block table swaps.
    import concourse.mybir as _mb
    _AF = _mb.ActivationFunctionType
    t = {k: set(v) for k, v in _orig_get_tables(arch).items()}
    for name, fs in t.items():
        if _AF.Exp in fs and _AF.Ln not in fs:
            fs.discard(_AF.Exp)
    return t


_hw.get_activation_tables = _patched_tables
import concourse.bass as bass
import concourse.bacc as bacc
import concourse.mybir as mybir
from concourse.bass import broadcast_tensor_aps
from concourse.bass_utils import run_bass_kernel_spmd
from concourse.tile import TileContext

F32 = mybir.dt.float32
BF16 = mybir.dt.bfloat16
U32 = mybir.dt.uint32
AF = mybir.ActivationFunctionType
OP = mybir.AluOpType
BF = ml_dtypes.bfloat16

C, L, DI, N, DTR = 64, 128, 128, 16, 4
SB = 4                   # sequences per block
TOK = SB * L             # 512
LAT = N * TOK            # 8192
NBLK = (L * L) // TOK    # 32 blocks per axis
HW = L * L
PADW = 4 + SB * 132
TW = PADW + 4
GN_EPS = 1e-5
NCORES = 8

import os as _os

SB = int(_os.environ.get("K_SB", 2))
TOK = SB * L
LAT = N * TOK
NBLK = (L * L) // TOK
PADW = 4 + SB * 132
TW = PADW + 4
POW_N0 = int(_os.environ.get("K_N0", 1))
PB = int(_os.environ.get("K_PB", 4))
PC = int(_os.environ.get("K_PC", 4))
PT = int(_os.environ.get("K_PT", 2))
LATBUFS = int(_os.environ.get("K_LATBUFS", 13))
B_DMA = int(_os.environ.get("K_BDMA", 0))
SILU_CHAIN = int(_os.environ.get("K_SILUCH", 0))
CCE_LVL1 = False
SIM_SAFE_SILU = False

_CACHE = {}
PROFILE = False


def _shv(tok, off):
    # shifted rhs view: (p, SB, L) reading cols off+132*s+t
    return tok[:, off:off + SB * 132].rearrange(
        "p (s u) -> p s u", u=132)[:, :, 0:L]


def _build():
    nc = bacc.Bacc()
    xsrc = [nc.dram_tensor("xrow", (C, HW), BF16, kind="ExternalInput"),
            nc.dram_tensor("xcol", (C, HW), BF16, kind="ExternalInput")]
    out = nc.dram_tensor("out", (C, HW), F32, kind="ExternalOutput")
    cscr = nc.dram_tensor("cscr", (8 * NBLK, LAT), BF16, kind="ExternalInput")

    pr = []
    for ax in ("r", "c"):
        pr.append(dict(
            W1=nc.dram_tensor(f"{ax}_W1", (2 * C, DI), BF16, kind="ExternalInput"),
            W2=nc.dram_tensor(f"{ax}_W2", (2 * C, DI), BF16, kind="ExternalInput"),
            Wz=nc.dram_tensor(f"{ax}_Wz", (C, DI), BF16, kind="ExternalInput"),
            Wd=nc.dram_tensor(f"{ax}_Wd", (DI, DI), BF16, kind="ExternalInput"),
            Wbc=nc.dram_tensor(f"{ax}_Wbc", (DI, 2 * N), BF16, kind="ExternalInput"),
            Wow=nc.dram_tensor(f"{ax}_Wow", (DI, C), BF16, kind="ExternalInput"),
            dtb=nc.dram_tensor(f"{ax}_dtb", (DI, 1), F32, kind="ExternalInput"),
            cvb=nc.dram_tensor(f"{ax}_cvb", (DI, 1), F32, kind="ExternalInput"),
            ncvb=nc.dram_tensor(f"{ax}_ncvb", (DI, 1), F32, kind="ExternalInput"),
            Dp=nc.dram_tensor(f"{ax}_Dp", (DI, 1), F32, kind="ExternalInput"),
        ))
    selg = nc.dram_tensor("selg", (C, 4), F32, kind="ExternalInput")
    selc = nc.dram_tensor("selc", (4, C), F32, kind="ExternalInput")
    gnw = nc.dram_tensor("gnw", (C, 1), F32, kind="ExternalInput")
    gnb = nc.dram_tensor("gnb", (C, 1), F32, kind="ExternalInput")

    with TileContext(nc) as tc:
        with ExitStack() as ctx:
            cpool = ctx.enter_context(tc.tile_pool(name="consts", bufs=1))
            tpool = ctx.enter_context(tc.tile_pool(name="tokp", bufs=1))
            spool = ctx.enter_context(tc.tile_pool(name="small", bufs=1))
            lpool = ctx.enter_context(tc.tile_pool(name="lat", bufs=1))
            xpool = ctx.enter_context(tc.tile_pool(name="xrec", bufs=1))
            pp = ctx.enter_context(tc.tile_pool(name="ps", bufs=1, space="PSUM"))

            cs = []
            for i, p in enumerate(pr):
                cs.append({k: cpool.tile_from(v[:], name=f"c{i}{k}")
                           for k, v in p.items()})
            selg_s = cpool.tile_from(selg[:], name="selg")
            selc_s = cpool.tile_from(selc[:], name="selc")
            gnw_s = cpool.tile_from(gnw[:], name="gnw")
            gnb_s = cpool.tile_from(gnb[:], name="gnb")

            xrec = xpool.tile([C, HW], BF16)
            xrv = xrec[:].rearrange("c (h w) -> c w h", w=L)

            GNINC = int(_os.environ.get("K_GNINC", 0))
            if GNINC:
                statsP = spool.tile([C, 2 * NBLK], F32, tag="statsP")

            for ax in (0, 1):
                kp = cs[ax]
                src = xsrc[ax]
                for blk in range(NBLK):
                    t0 = blk * TOK
                    src3 = src[:, t0:t0 + TOK].rearrange(
                        "c (s t) -> c s t", t=L)
                    tokF = tpool.tile([2 * C, TW], BF16, tag="tokF", bufs=2)
                    tokR = tpool.tile([2 * C, TW], BF16, tag="tokR", bufs=2)
                    nc.vector.memzero(tokF[:])
                    nc.vector.memzero(tokR[:])
                    for tok, boff in ((tokF, 5), (tokR, 3)):
                        nc.sync.dma_start(
                            tok[0:C, 4:PADW].rearrange(
                                "p (s u) -> p s u", u=132)[:, :, 0:L], src3)
                        nc.sync.dma_start(
                            tok[C:2 * C, boff:boff + SB * 132].rearrange(
                                "p (s u) -> p s u", u=132)[:, :, 0:L], src3)

                    # x-half conv-fused matmuls + z-half
                    ps_x = [pp.tile([DI, TOK], F32, tag="psx", bufs=2,
                                    name=f"psx{_}") for _ in range(2)]
                    nc.tensor.matmul(ps_x[0][:], kp["W1"][:], _shv(tokF, 4),
                                     start=True, stop=False)
                    nc.tensor.matmul(ps_x[0][:], kp["W2"][:], _shv(tokF, 2),
                                     start=False, stop=True)
                    nc.tensor.matmul(ps_x[1][:], kp["W1"][:], _shv(tokR, 4),
                                     start=True, stop=False)
                    nc.tensor.matmul(ps_x[1][:], kp["W2"][:], _shv(tokR, 6),
                                     start=False, stop=True)
                    ps_z = pp.tile([DI, TOK], F32, tag="psz", bufs=1)
                    nc.tensor.matmul(ps_z[:], kp["Wz"][:],
                                     _shv(tokF, 4)[0:C], start=True, stop=True)

                    xt = [spool.tile([DI, TOK], BF16, tag=f"xt{d}", bufs=2,
                                     name=f"xt{d}") for d in range(2)]
                    if SILU_CHAIN and not SIM_SAFE_SILU:
                        # silu via exp/ln chain (single act table)
                        for d in range(2):
                            e1 = spool.tile([DI, TOK], F32, tag="e1", bufs=2)
                            nc.scalar.activation(e1[:], ps_x[d][:], AF.Exp,
                                                 scale=-1.0,
                                                 bias=kp["ncvb"][:])
                            sp1 = spool.tile([DI, TOK], F32, tag="sp1",
                                             bufs=2)
                            nc.scalar.activation(sp1[:], e1[:], AF.Ln,
                                                 bias=1.0)
                            sg = spool.tile([DI, TOK], BF16, tag="sg",
                                            bufs=2)
                            nc.scalar.activation(sg[:], sp1[:], AF.Exp,
                                                 scale=-1.0)
                            nc.vector.scalar_tensor_tensor(
                                xt[d][:], ps_x[d][:], kp["cvb"][:], sg[:],
                                OP.add, OP.mult)
                        zs = spool.tile([DI, TOK], BF16, tag="zs", bufs=2)
                        e1z = spool.tile([DI, TOK], F32, tag="e1", bufs=2)
                        nc.scalar.activation(e1z[:], ps_z[:], AF.Exp,
                                             scale=-1.0)
                        sp1z = spool.tile([DI, TOK], F32, tag="sp1", bufs=2)
                        nc.scalar.activation(sp1z[:], e1z[:], AF.Ln,
                                             bias=1.0)
                        sgz = spool.tile([DI, TOK], BF16, tag="sg", bufs=2)
                        nc.scalar.activation(sgz[:], sp1z[:], AF.Exp,
                                             scale=-1.0)
                        nc.vector.tensor_tensor(zs[:], ps_z[:], sgz[:],
                                                OP.mult)
                    else:
                        for d in range(2):
                            if SIM_SAFE_SILU:
                                vv = spool.tile([DI, TOK], F32, tag="vv",
                                                bufs=2)
                                nc.scalar.activation(vv[:], ps_x[d][:],
                                                     AF.Identity,
                                                     bias=kp["cvb"][:])
                                nc.scalar.activation(xt[d][:], ps_x[d][:],
                                                     AF.Sigmoid,
                                                     bias=kp["cvb"][:])
                                nc.vector.tensor_tensor(xt[d][:], xt[d][:],
                                                        vv[:], OP.mult)
                            else:
                                nc.scalar.activation(xt[d][:], ps_x[d][:],
                                                     AF.Silu,
                                                     bias=kp["cvb"][:])
                        zs = spool.tile([DI, TOK], BF16, tag="zs", bufs=2)
                        if SIM_SAFE_SILU:
                            nc.scalar.activation(zs[:], ps_z[:], AF.Sigmoid)
                            nc.vector.tensor_tensor(zs[:], zs[:], ps_z[:],
                                                    OP.mult)
                        else:
                            nc.scalar.activation(zs[:], ps_z[:], AF.Silu)

                    ps_d = [pp.tile([DI, TOK], F32, tag="psd", bufs=2,
                                    name=f"psd{_}") for _ in range(2)]
                    ps_bc = [pp.tile([2 * N, TOK], F32, tag="psbc", bufs=2,
                                     name=f"psbc{_}") for _ in range(2)]
                    for d in range(2):
                        nc.tensor.matmul(ps_d[d][:], kp["Wd"][:], xt[d][:],
                                         start=True, stop=True)
                        nc.tensor.matmul(ps_bc[d][:], kp["Wbc"][:], xt[d][:],
                                         start=True, stop=True)

                    dA = [lpool.tile([DI, LAT], BF16, tag="lat", bufs=LATBUFS,
                                     name=f"dA{d}") for d in range(2)]
                    delta = [spool.tile([DI, TOK], BF16, tag=f"dl{d}", bufs=2,
                                        name=f"dl{d}") for d in range(2)]
                    bcsb = [spool.tile([2 * N, TOK], BF16, tag=f"bc{d}", bufs=2,
                                       name=f"bc{d}") for d in range(2)]
                    # [exp/ln table]
                    for d in range(2):
                        esb = spool.tile([DI, TOK], F32, tag="esb", bufs=2)
                        nc.scalar.activation(esb[:], ps_d[d][:], AF.Exp,
                                             bias=kp["dtb"][:])
                        nc.scalar.activation(delta[d][:], esb[:], AF.Ln,
                                             bias=1.0)
                        nc.scalar.activation(dA[d][:, 0:TOK], delta[d][:],
                                             AF.Exp, scale=-1.0)
                        for n_ in range(POW_N0, N):
                            nc.scalar.activation(
                                dA[d][:, n_ * TOK:(n_ + 1) * TOK],
                                delta[d][:], AF.Exp, scale=-float(n_ + 1))
                        nc.scalar.activation(bcsb[d][:], ps_bc[d][:], AF.Copy)

                    for d in range(2):
                        # powers: dA[n] = q^(n+1), log-depth chain on DVE
                        # for slots [1, POW_N0); Act exp handles the rest
                        q = dA[d][:, 0:TOK]
                        if POW_N0 > 1:
                            nc.vector.tensor_tensor(dA[d][:, TOK:2 * TOK],
                                                    q, q, OP.mult)
                        lo = 2
                        while lo < POW_N0:
                            hi = min(2 * lo, POW_N0)
                            w_ = hi - lo
                            dst = dA[d][:, lo * TOK:hi * TOK].rearrange(
                                "p (n t) -> p n t", t=TOK)
                            sa = dA[d][:, 0:w_ * TOK].rearrange(
                                "p (n t) -> p n t", t=TOK)
                            sb_, _ = broadcast_tensor_aps(
                                dA[d][:, (lo - 1) * TOK:lo * TOK].rearrange(
                                    "p (o t) -> p o t", o=1), dst)
                            nc.vector.tensor_tensor(dst, sa, sb_, OP.mult)
                            lo = hi

                        du = spool.tile([DI, TOK], BF16, tag=f"du{d}", bufs=2)
                        nc.vector.tensor_tensor(du[:], delta[d][:], xt[d][:],
                                                OP.mult)

                        row = (2 * ax + d) * NBLK + blk
                        nc.sync.dma_start(
                            cscr[row:row + 1, :].rearrange(
                                "o (n t) -> o n t", t=TOK),
                            bcsb[d][N:2 * N, :])
                        Bb = lpool.tile([DI, LAT], BF16, tag="lat", bufs=LATBUFS)
                        if B_DMA:
                            browe = 4 * NBLK + row
                            nc.sync.dma_start(
                                cscr[browe:browe + 1, :].rearrange(
                                    "o (n t) -> o n t", t=TOK),
                                bcsb[d][0:N, :])
                            bin_, _ = broadcast_tensor_aps(
                                cscr[browe:browe + 1, :].rearrange(
                                    "o f -> o () f"),
                                Bb[:].rearrange("p f -> p () f"))
                            nc.sync.dma_start(
                                Bb[:].rearrange("p f -> p () f"), bin_)
                        else:
                            bflat = spool.tile([1, LAT], BF16, tag="bf",
                                               bufs=2)
                            nc.sync.dma_start(bflat[:].rearrange(
                                "o (n t) -> o n t", t=TOK), bcsb[d][0:N, :])
                            nc.gpsimd.partition_broadcast(
                                Bb[:].bitcast(U32), bflat[:].bitcast(U32))
                        Cb = lpool.tile([DI, LAT], BF16, tag="lat", bufs=LATBUFS)
                        cin, _ = broadcast_tensor_aps(
                            cscr[row:row + 1, :].rearrange("o f -> o () f"),
                            Cb[:].rearrange("p f -> p () f"))
                        nc.sync.dma_start(Cb[:].rearrange("p f -> p () f"),
                                          cin)

                        # dBu = Bb * du (broadcast over n), in place,
                        # split between Pool (n<PB) and DVE
                        if PB > 0:
                            dBp = Bb[:, 0:PB * TOK].rearrange(
                                "p (n t) -> p n t", t=TOK)
                            dup, _ = broadcast_tensor_aps(
                                du[:].rearrange("p (o t) -> p o t", o=1), dBp)
                            nc.gpsimd.tensor_tensor(dBp, dBp, dup, OP.mult)
                        dB3 = Bb[:, PB * TOK:].rearrange(
                            "p (n t) -> p n t", t=TOK)
                        du3, _ = broadcast_tensor_aps(
                            du[:].rearrange("p (o t) -> p o t", o=1), dB3)
                        nc.vector.tensor_tensor(dB3, dB3, du3, OP.mult)

                        # zero dA at segment boundaries, then scan
                        dA4 = dA[d][:].rearrange("p (g t) -> p g t", t=L)
                        if d == 0:
                            nc.vector.memset(dA4[:, :, 0:1], 0.0)
                        else:
                            nc.vector.memset(dA4[:, :, L - 1:L], 0.0)
                        h = lpool.tile([DI, LAT], BF16, tag="lat", bufs=LATBUFS)
                        if d == 0:
                            nc.vector.tensor_tensor_scan(
                                h[:], dA[d][:], Bb[:], 0.0, OP.mult, OP.add)
                        else:
                            nc.vector.tensor_tensor_scan(
                                h[:, ::-1], dA[d][:, ::-1], Bb[:, ::-1], 0.0,
                                OP.mult, OP.add)

                        # hC in place (Pool takes n<PC), then tree reduce
                        if PC > 0:
                            nc.gpsimd.tensor_tensor(
                                h[:, 0:PC * TOK], h[:, 0:PC * TOK],
                                Cb[:, 0:PC * TOK], OP.mult)
                        nc.vector.tensor_tensor(h[:, PC * TOK:],
                                                h[:, PC * TOK:],
                                                Cb[:, PC * TOK:], OP.mult)
                        if PT > 0:
                            nc.gpsimd.tensor_tensor(
                                h[:, 0:PT * TOK], h[:, 0:PT * TOK],
                                h[:, LAT // 2:LAT // 2 + PT * TOK], OP.add)
                        nc.vector.tensor_tensor(
                            h[:, PT * TOK:LAT // 2], h[:, PT * TOK:LAT // 2],
                            h[:, LAT // 2 + PT * TOK:LAT], OP.add)
                        for q_ in (4, 2):
                            nc.vector.tensor_tensor(
                                h[:, 0:q_ * TOK], h[:, 0:q_ * TOK],
                                h[:, q_ * TOK:2 * q_ * TOK], OP.add)
                        y = spool.tile([DI, TOK], BF16, tag=f"y{d}", bufs=2)
                        nc.vector.tensor_tensor(y[:], h[:, 0:TOK],
                                                h[:, TOK:2 * TOK], OP.add)
                        if d == 0:
                            y0 = y
                    # combine dirs: y2 = (ysum + Dp*xtsum) * zs
                    xts = spool.tile([DI, TOK], BF16, tag="xts", bufs=2)
                    nc.vector.tensor_tensor(xts[:], xt[0][:], xt[1][:], OP.add)
                    ys = spool.tile([DI, TOK], BF16, tag="ys", bufs=2)
                    nc.vector.tensor_tensor(ys[:], y0[:], y[:], OP.add)
                    y2 = spool.tile([DI, TOK], BF16, tag="y2", bufs=2)
                    nc.vector.scalar_tensor_tensor(
                        y2[:], xts[:], kp["Dp"][:], ys[:], OP.mult, OP.add)
                    nc.vector.tensor_tensor(y2[:], y2[:], zs[:], OP.mult)

                    ps_o = pp.tile([C, TOK], F32, tag="pso", bufs=1)
                    nc.tensor.matmul(ps_o[:], kp["Wow"][:], y2[:],
                                     start=True, stop=True)
                    if ax == 0:
                        nc.vector.tensor_copy(xrec[:, t0:t0 + TOK], ps_o[:])
                    else:
                        dst = xrv[:, SB * blk:SB * (blk + 1), :]
                        nc.vector.tensor_tensor(
                            dst, dst,
                            ps_o[:].rearrange("c (s t) -> c s t", t=L),
                            OP.add)
                        if GNINC:
                            # columns of this w-slice are final: fold their
                            # sum/sumsq into the running stats now
                            nc.vector.tensor_reduce(
                                statsP[:, blk:blk + 1], dst,
                                mybir.AxisListType.XY, OP.add)
                            sqb = spool.tile([C, TOK], F32, tag="sqb",
                                             bufs=2)
                            nc.gpsimd.tensor_tensor(
                                sqb[:].rearrange("c (s t) -> c s t", t=L),
                                dst, dst, OP.mult)
                            nc.vector.tensor_reduce(
                                statsP[:, NBLK + blk:NBLK + blk + 1],
                                sqb[:], mybir.AxisListType.X, OP.add)

            # GroupNorm(4) + SiLU + residual
            NCH = 8
            CHK = HW // NCH
            st2 = spool.tile([C, 2], F32, tag="st2")
            if GNINC:
                nc.vector.tensor_reduce(
                    st2[:], statsP[:].rearrange("c (a j) -> c a j", a=2),
                    mybir.AxisListType.X, OP.add)
            else:
                stats = spool.tile([C, 2 * NCH], F32, tag="stats")
                for j in range(NCH):
                    ch = xrec[:, j * CHK:(j + 1) * CHK]
                    nc.vector.tensor_reduce(stats[:, j:j + 1], ch,
                                            mybir.AxisListType.X, OP.add)
                    sq = lpool.tile([C, CHK], F32, tag="lat", bufs=LATBUFS)
                    nc.scalar.activation(
                        sq[:], ch, AF.Square,
                        accum_out=stats[:, NCH + j:NCH + j + 1])
                nc.vector.tensor_reduce(
                    st2[:], stats[:].rearrange("c (a j) -> c a j", a=2),
                    mybir.AxisListType.X, OP.add)
            ps_g = pp.tile([4, 2], F32, tag="pso")
            nc.tensor.matmul(ps_g[:], selg_s[:], st2[:], start=True, stop=True)
            mv = spool.tile([4, 2], F32, tag="mv")
            nc.vector.tensor_scalar_mul(mv[:], ps_g[:], 1.0 / (16 * HW))
            mu = mv[:, 0:1]
            var = spool.tile([4, 1], F32, tag="var")
            nc.vector.tensor_tensor(var[:], mu, mu, OP.mult)
            nc.vector.tensor_tensor(var[:], mv[:, 1:2], var[:], OP.subtract)
            sd = spool.tile([4, 1], F32, tag="sd")
            nc.vector.tensor_scalar_add(var[:], var[:], GN_EPS)
            nc.scalar.activation(sd[:], var[:], AF.Sqrt)
            rs = spool.tile([4, 1], F32, tag="rs")
            nc.vector.reciprocal(rs[:], sd[:])
            murs = spool.tile([4, 2], F32, tag="mv2")
            nc.vector.tensor_copy(murs[:, 0:1], mu)
            nc.vector.tensor_copy(murs[:, 1:2], rs[:])
            ps_c = pp.tile([C, 2], F32, tag="pso")
            nc.tensor.matmul(ps_c[:], selc_s[:], murs[:], start=True, stop=True)
            aa = spool.tile([C, 1], F32, tag="aa")
            nc.vector.tensor_tensor(aa[:], ps_c[:, 1:2], gnw_s[:], OP.mult)
            bb = spool.tile([C, 1], F32, tag="bb")
            nc.vector.tensor_tensor(bb[:], ps_c[:, 0:1], aa[:], OP.mult)
            nc.vector.tensor_tensor(bb[:], gnb_s[:], bb[:], OP.subtract)
            for j in range(NCH):
                sil = lpool.tile([C, CHK], F32, tag="lat", bufs=LATBUFS)
                if SIM_SAFE_SILU:
                    vv2 = lpool.tile([C, CHK], F32, tag="lat", bufs=LATBUFS)
                    nc.scalar.activation(vv2[:],
                                         xrec[:, j * CHK:(j + 1) * CHK],
                                         AF.Identity, scale=aa[:],
                                         bias=bb[:])
                    nc.scalar.activation(sil[:],
                                         xrec[:, j * CHK:(j + 1) * CHK],
                                         AF.Sigmoid, scale=aa[:],
                                         bias=bb[:])
                    nc.vector.tensor_tensor(sil[:], sil[:], vv2[:], OP.mult)
                else:
                    nc.scalar.activation(sil[:],
                                         xrec[:, j * CHK:(j + 1) * CHK],
                                         AF.Silu, scale=aa[:], bias=bb[:])
                xres = lpool.tile([C, CHK], BF16, tag="lat", bufs=LATBUFS)
                nc.sync.dma_start(xres[:], xsrc[0][:, j * CHK:(j + 1) * CHK])
                nc.vector.tensor_tensor(sil[:], sil[:], xres[:], OP.add)
                nc.sync.dma_start(out[:, j * CHK:(j + 1) * CHK], sil[:])
    nc.compile()
    return nc


def _prep(axp):
    in_w, conv_w, conv_b, xp_w, dt_w, dt_b, A_log, Dp, out_w = [
        np.asarray(v, np.float64) for v in axp]
    wx = in_w[:DI, :]                        # (DI, C)
    wz = in_w[DI:2 * DI, :]
    wk = conv_w[:, 0, :]                     # (DI, 4) taps
    d = {}
    d["W1"] = np.concatenate([wx.T * wk[:, 3], wx.T * wk[:, 2]], 0)
    d["W2"] = np.concatenate([wx.T * wk[:, 1], wx.T * wk[:, 0]], 0)
    d["Wz"] = wz.T
    d["Wd"] = xp_w[:DTR].T @ dt_w.T          # (DI, DI)
    d["Wbc"] = xp_w[DTR:].T                  # (DI, 2N) B then C
    d["Wow"] = 0.25 * out_w.T
    d = {k: np.ascontiguousarray(v.astype(BF)) for k, v in d.items()}
    d["dtb"] = dt_b.astype(np.float32).reshape(DI, 1)
    d["cvb"] = conv_b.astype(np.float32).reshape(DI, 1)
    d["ncvb"] = (-conv_b).astype(np.float32).reshape(DI, 1)
    d["Dp"] = Dp.astype(np.float32).reshape(DI, 1)
    A = -np.exp(A_log)
    assert np.allclose(A, -np.arange(1., N + 1.)[None, :], atol=1e-4), \
        "kernel assumes A[d,n] = -(n+1)"
    return d


def kernel(**inputs):
    x = np.asarray(inputs["x"], np.float32)
    b = x.shape[0]
    names = ("in_w", "conv_w", "conv_b", "xp_w", "dt_w", "dt_b", "A_log",
             "D", "out_w")
    rp = _prep([inputs["row_" + n] for n in names])
    cp = _prep([inputs["col_" + n] for n in names])

    if "nc" not in _CACHE:
        _CACHE["nc"] = _build()
    nc = _CACHE["nc"]

    base = {}
    for k, v in rp.items():
        base["r_" + k] = v
    for k, v in cp.items():
        base["c_" + k] = v
    selg = np.zeros((C, 4), np.float32)
    for c in range(C):
        selg[c, c // 16] = 1.0
    base["selg"] = selg
    base["selc"] = np.ascontiguousarray(selg.T)
    base["gnw"] = np.asarray(inputs["gn_w"], np.float32).reshape(C, 1)
    base["gnb"] = np.asarray(inputs["gn_b"], np.float32).reshape(C, 1)

    in_maps = []
    for i in range(NCORES):
        m = dict(base)
        xi = x[i % b]                          # (C, L, L)
        m["xrow"] = np.ascontiguousarray(xi.reshape(C, HW).astype(BF))
        m["xcol"] = np.ascontiguousarray(
            xi.transpose(0, 2, 1).reshape(C, HW).astype(BF))
        m["cscr"] = np.zeros((8 * NBLK, LAT), BF)
        in_maps.append(m)
    res = run_bass_kernel_spmd(nc, in_maps, list(range(NCORES)),
                               trace=PROFILE)
    if PROFILE and res.exec_time_ns is not None:
        print(f"HW exec time: {res.exec_time_ns} ns")
        _CACHE["exec_time_ns"] = res.exec_time_ns
    outs = [res.results[i]["out"].reshape(C, L, L) for i in range(b)]
    return np.stack(outs, 0).astype(np.float32)

